# revision 1
# baseline (speedup 1.0000x reference)
"""ConvR (dense_cnn) Trainium2 kernel — 8-core vocab/tensor-parallel.

Strategy (per sharding hint): the entity-embedding table and output scores are
column-sharded across the 8 cores; the small conv/fc path is replicated on
every core (each core computes the full 256-sample hidden, then scores its
12500-entity shard).

Host side: gathers (emb_e[e1], emb_rel[rel]), BatchNorm constant-folding into
affine scale/shift, and data layout so every device matmul has its contraction
dim on partitions:
  - conv: per-sample matmul  x_b(100c,36hw) = filtersT_b(25k,100c).T @ patches_b(25k,36hw)
  - fc:   accumulate over hw: hT(100j,256b) += W_hw(100c,100j).T @ X_hw(100c,256b)
  - scoring: scores(128b,500e) = hT_aug(101,128b).T @ embT_aug(101,500e)
    (bias folded in as row 100 of embT with a ones-row in hT)
"""
import os
import sys

sys.path.insert(0, "/opt/trn_rl_repo")

import numpy as np
from contextlib import ExitStack

B = 256          # batch
E = 100          # embedding dim
NE = 100000      # entities
NCORES = 8
SH = NE // NCORES   # 12500 entities per core
NCH = 500           # scoring N-chunk (one PSUM bank, >=256 for f32r full rate)
NS = 14             # conv samples per PSUM tile (14*36=504 <= 512)
EPS = 1e-5

_CACHE = {}


def _build(use_f32r=True):
    import concourse.bass as bass  # noqa: F401
    import concourse.tile as tile
    from concourse import bacc, mybir

    f32 = mybir.dt.float32
    # float32r: same fp32 bits, PE streams at 1 cycle/row for N>=256 (vs 4
    # for plain fp32). The BIR verifier requires every producer feeding an
    # f32r matmul to be typed f32r, so the fc/scoring operand tensors (w3,
    # embT, X, hT, ones) are declared f32r end-to-end.
    fr = mybir.dt.float32r if use_f32r else f32
    AF = mybir.ActivationFunctionType
    OP = mybir.AluOpType

    nc = bacc.Bacc("TRN2", target_bir_lowering=False, debug=False,
                   num_devices=NCORES)

    # all conv operands at base partition 0 (PE misbehaves when consecutive
    # matmuls switch lhsT base partition); stream r3/p3 in CH-sample chunks
    CH = 32
    r3_d = nc.dram_tensor("r3", [25, B * 100], fr, kind="ExternalInput").ap()
    p3_d = nc.dram_tensor("p3", [25, B * 36], fr, kind="ExternalInput").ap()
    b1_d = nc.dram_tensor("b1c", [100, 1], f32, kind="ExternalInput").ap()
    w3_d = nc.dram_tensor("w3", [100, 3600], fr, kind="ExternalInput").ap()
    b2_d = nc.dram_tensor("b2c", [100, 1], f32, kind="ExternalInput").ap()
    ones_d = nc.dram_tensor("ones", [1, B], fr, kind="ExternalInput").ap()
    embT_d = nc.dram_tensor("embT", [101, SH], fr, kind="ExternalInput").ap()
    scores_d = nc.dram_tensor("scores", [B, SH], f32, kind="ExternalOutput").ap()

    with tile.TileContext(nc) as tc, ExitStack() as ctx:
        cpool = ctx.enter_context(tc.tile_pool(name="const", bufs=1))

        def load(dram_ap, shape, tag, dt=f32, eng=None):
            t = cpool.tile(shape, dt, tag=tag)
            (eng or nc.sync).dma_start(t[:], dram_ap[:])
            return t

        b1_t = load(b1_d, [100, 1], "b1c")
        w3_t = load(w3_d, [100, 3600], "w3", fr, eng=nc.gpsimd)
        b2_t = load(b2_d, [100, 1], "b2c")
        embT_t = load(embT_d, [101, SH], "embT", fr, eng=nc.scalar)

        # conv: per-sample matmuls, evacuate relu(x + B1) into X[c, hw*B + s]
        X_t = cpool.tile([100, 36 * B], fr, tag="X")
        rpool = ctx.enter_context(tc.tile_pool(name="rch", bufs=3))
        ppool = ctx.enter_context(tc.tile_pool(name="pch", bufs=3))
        pconv = ctx.enter_context(tc.tile_pool(name="pconv", bufs=2, space="PSUM"))
        Xv = X_t[:].rearrange("p (hw s) -> p s hw", s=B)
        rc = pc = None
        ntile = (B + NS - 1) // NS
        for it in range(ntile):
            s0 = it * NS
            n = min(NS, B - s0)
            pt = pconv.tile([100, NS * 36], f32, tag="pconv")
            for i in range(n):
                s = s0 + i
                c, off = divmod(s, CH)
                if off == 0:
                    rc = rpool.tile([25, CH * 100], fr, tag="rch")
                    nc.sync.dma_start(rc[:], r3_d[:, c * CH * 100:(c + 1) * CH * 100])
                    pc = ppool.tile([25, CH * 36], fr, tag="pch")
                    nc.sync.dma_start(pc[:], p3_d[:, c * CH * 36:(c + 1) * CH * 36])
                nc.tensor.matmul(
                    pt[:, i * 36:(i + 1) * 36],
                    rc[:, off * 100:(off + 1) * 100],
                    pc[:, off * 36:(off + 1) * 36],
                    start=True, stop=True)
            src = pt[:, 0:n * 36].rearrange("p (s hw) -> p s hw", hw=36)
            nc.scalar.activation(Xv[:, s0:s0 + n, :], src, AF.Relu,
                                 bias=b1_t[:, 0:1])

        # fc: accumulate 36 matmuls into one PSUM tile
        pfc_pool = ctx.enter_context(tc.tile_pool(name="pfc", bufs=1, space="PSUM"))
        pfc = pfc_pool.tile([100, B], f32, tag="pfc")
        for hw in range(36):
            nc.tensor.matmul(
                pfc[:],
                w3_t[:, hw * 100:(hw + 1) * 100],
                X_t[:, hw * B:(hw + 1) * B],
                start=(hw == 0), stop=(hw == 35))
        hT_t = cpool.tile([101, B], fr, tag="hT")
        nc.scalar.activation(hT_t[0:100, :], pfc[:], AF.Relu, bias=b2_t[:, 0:1])
        nc.sync.dma_start(hT_t[100:101, :], ones_d[:])

        # scoring: scores[m*128:+128, ci*500:+500] = sigmoid(hT_aug.T @ embT_aug)
        psc = ctx.enter_context(tc.tile_pool(name="psc", bufs=4, space="PSUM"))
        sbp = ctx.enter_context(tc.tile_pool(name="sb", bufs=4))
        for m in range(B // 128):
            for ci in range(SH // NCH):
                ps = psc.tile([128, NCH], f32, tag="psc")
                nc.tensor.matmul(
                    ps[:],
                    hT_t[:, m * 128:(m + 1) * 128],
                    embT_t[:, ci * NCH:(ci + 1) * NCH],
                    start=True, stop=True)
                sb = sbp.tile([128, NCH], f32, tag="sb")
                nc.scalar.activation(sb[:], ps[:], AF.Sigmoid)
                out_engs = (nc.sync, nc.gpsimd)
                out_engs[(m * (SH // NCH) + ci) % 2].dma_start(
                    scores_d[m * 128:(m + 1) * 128, ci * NCH:(ci + 1) * NCH],
                    sb[:])

    nc.compile()
    return nc


def host_prep(inputs):
    f = {k: np.asarray(v) for k, v in inputs.items()}
    e1 = f['e1'].astype(np.int64)
    rel = f['rel'].astype(np.int64)
    e1e = np.ascontiguousarray(f['emb_e'][e1]).astype(np.float32)    # (B, 100)
    rg = np.ascontiguousarray(f['emb_rel'][rel]).astype(np.float32)  # (B, 2500)

    a0 = float(f['bn0_g'][0] / np.sqrt(f['bn0_v'][0] + EPS))
    b0 = float(f['bn0_b'][0] - f['bn0_m'][0] * a0)
    A1 = (f['bn1_g'] / np.sqrt(f['bn1_v'] + EPS)).astype(np.float32)
    B1 = (f['bn1_b'] - f['bn1_m'] * A1).astype(np.float32)
    s_rel = (f['bn_rel_g'] / np.sqrt(f['bn_rel_v'] + EPS)).astype(np.float32)
    t_rel = (f['bn_rel_b'] - f['bn_rel_m'] * s_rel).astype(np.float32)
    s_rel2 = s_rel * np.repeat(A1, 25)
    t_rel2 = t_rel * np.repeat(A1, 25)
    A2 = (f['bn2_g'] / np.sqrt(f['bn2_v'] + EPS)).astype(np.float32)
    B2p = ((f['fc_b'] - f['bn2_m']) * A2 + f['bn2_b']).astype(np.float32)

    # normalized, A1-folded filters in k-on-partition layout:
    # r3[k, s*100+c] = (rg*s_rel2 + t_rel2)[s, c*25+k]
    rn = rg * s_rel2[None, :] + t_rel2[None, :]
    r3 = np.ascontiguousarray(
        rn.reshape(B, 100, 25).transpose(2, 0, 1).reshape(25, B * 100))
    # BN0-normalized patches: p3[k, s*36+hw] = x0[s, patch(k, hw)]
    x0 = e1e * a0 + b0
    xg = x0.reshape(B, 10, 10)
    win = np.lib.stride_tricks.sliding_window_view(xg, (5, 5), axis=(1, 2))
    p3 = np.ascontiguousarray(
        win.transpose(3, 4, 0, 1, 2).reshape(25, B * 36))
    w3 = np.ascontiguousarray(
        (f['fc_w'].astype(np.float32) * A2[None, :]).reshape(100, 3600))
    embT = np.ascontiguousarray(np.concatenate(
        [f['emb_e'].T, f['bias'][None, :]], 0).astype(np.float32))  # (101, NE)

    col = lambda v: np.ascontiguousarray(v.reshape(100, 1)).astype(np.float32)
    common = dict(
        r3=r3.astype(np.float32), p3=p3.astype(np.float32),
        b1c=col(B1), w3=w3.astype(np.float32), b2c=col(B2p),
        ones=np.ones((1, B), np.float32))
    in_maps = []
    for m in range(NCORES):
        d = dict(common)
        d['embT'] = np.ascontiguousarray(embT[:, m * SH:(m + 1) * SH])
        in_maps.append(d)
    return in_maps


def _get_nc():
    if 'nc' not in _CACHE:
        _CACHE['nc'] = _build(use_f32r=False)
    return _CACHE['nc']


def kernel(**inputs):
    from concourse import bass_utils
    from concourse.bass_interp import get_hw_module

    nc = _get_nc()
    in_maps = host_prep(inputs)

    kwargs = {}
    trace_dir = os.environ.get("CONVR_TRACE_DIR")
    if trace_dir:
        kwargs.update(tmpdir=trace_dir, trace=True)

    old_m = nc.m
    nc.m = get_hw_module(nc.m)
    try:
        res = bass_utils.run_bass_kernel_spmd(
            nc, in_maps, core_ids=list(range(NCORES)), **kwargs)
    finally:
        nc.m = old_m
    _CACHE['last_result'] = res

    out = np.empty((B, NE), np.float32)
    for m in range(NCORES):
        out[:, m * SH:(m + 1) * SH] = res.results[m]['scores']
    return out



# revision 6
# speedup vs baseline: 2.7841x; 2.7841x over previous
"""ConvR (dense_cnn) Trainium2 kernel — 8-core vocab/tensor-parallel, fp16.

Strategy (per sharding hint): entity-embedding table and output scores are
column-sharded across the 8 cores; the small conv/fc path is replicated on
every core (each core computes the full 256-sample hidden, then scores its
12500-entity shard).

v2 redesign vs baseline (384us):
  - all matmul operands fp16 (PE streams 1 cyc/col vs 4 for fp32, single
    LDWEIGHTS pass); PSUM accumulation stays f32.
  - conv packs S=4 samples per matmul via block-diagonal rhs: partitions
    hold 4x25 patch rows, rhs [100, 144] has each sample's 36 patch columns
    in its own 25-partition band (zeros elsewhere), lhsT [100, 100] stacks
    the 4 samples' filters. 64 matmuls instead of 256.
  - input DMAs chunked across queues/engines (a single dma_start tends to
    pin one ~22.5 GB/s DMA engine; the baseline's monolithic embT load
    serialized 5 MB onto one engine for 195us).
  - PSUM evacuation split between scalar(ACT) and vector(DVE) — the only
    two engines with PSUM access.
  - scores leave the device as fp16 logits; sigmoid + f32 upcast on host.
"""
import os
import sys

sys.path.insert(0, "/opt/trn_rl_repo")

import numpy as np
from contextlib import ExitStack

B = 256          # batch
E = 100          # embedding dim
NE = 100000      # entities
NCORES = 8
SH = NE // NCORES   # 12500 entities per core
NCH = 500           # scoring N-chunk (one PSUM bank of f32)
S = 4               # conv samples packed per matmul (4*25=100 partitions)
NG = B // S         # 64 conv groups
GC = S * 36         # 144 rhs cols per conv group
EPS = 1e-5

# chunk boundaries (in conv groups) for r5/p5 input streaming: small first
# chunks so the first conv matmul can start early
CONV_CHUNKS = [(0, 4), (4, 8), (8, 16), (16, 32), (32, 64)]

_CACHE = {}


def _emb_row_chunks():
    # 101 rows -> 13 partition chunks (12x8 + 5)
    bounds = list(range(0, 96, 8)) + [96, 101]
    return list(zip(bounds[:-1], bounds[1:]))


def _build():
    import concourse.bass as bass  # noqa: F401
    import concourse.tile as tile
    from concourse import bacc, mybir

    f32 = mybir.dt.float32
    f16 = mybir.dt.float16
    AF = mybir.ActivationFunctionType
    OP = mybir.AluOpType

    nc = bacc.Bacc("TRN2", target_bir_lowering=False, debug=False,
                   num_devices=NCORES)

    r5_d = nc.dram_tensor("r5", [100, NG * 100], f16, kind="ExternalInput").ap()
    p5_d = nc.dram_tensor("p5", [100, NG * GC], f16, kind="ExternalInput").ap()
    b1_d = nc.dram_tensor("b1c", [100, 1], f32, kind="ExternalInput").ap()
    w3_d = nc.dram_tensor("w3", [100, 3600], f16, kind="ExternalInput").ap()
    b2_d = nc.dram_tensor("b2c", [100, 1], f32, kind="ExternalInput").ap()
    ones_d = nc.dram_tensor("ones", [1, B], f16, kind="ExternalInput").ap()
    embT_d = nc.dram_tensor("embT", [101, SH], f16, kind="ExternalInput").ap()
    scores_d = nc.dram_tensor("scores", [B, SH], f16, kind="ExternalOutput").ap()

    with tile.TileContext(nc) as tc, ExitStack() as ctx:
        cpool = ctx.enter_context(tc.tile_pool(name="const", bufs=1))

        # biases first (tiny, needed at first conv evac)
        b1_t = cpool.tile([100, 1], f32, tag="b1c")
        nc.scalar.dma_start(b1_t[:], b1_d[:])
        b2_t = cpool.tile([100, 1], f32, tag="b2c")
        nc.scalar.dma_start(b2_t[:], b2_d[:])

        # embT: 13 partition-row chunks, all on the scalar queue (separate
        # dma_starts spread across DMA engines; only SP/ACT/gpsimd can issue)
        embT_t = cpool.tile([101, SH], f16, tag="embT")
        for r0, r1 in _emb_row_chunks():
            nc.scalar.dma_start(embT_t[r0:r1, :], embT_d[r0:r1, :])

        # conv operands: separate tiles per chunk (per-tile dep granularity),
        # alternating sync/gpsimd queues
        r5_ts, p5_ts = [], []
        for i, (g0, g1) in enumerate(CONV_CHUNKS):
            rt = cpool.tile([100, (g1 - g0) * 100], f16, tag=f"r5_{i}")
            pt_ = cpool.tile([100, (g1 - g0) * GC], f16, tag=f"p5_{i}")
            qa, qb = (nc.sync, nc.gpsimd) if i % 2 == 0 else (nc.gpsimd, nc.sync)
            qa.dma_start(rt[:], r5_d[:, g0 * 100:g1 * 100])
            qb.dma_start(pt_[:], p5_d[:, g0 * GC:g1 * GC])
            r5_ts.append(rt)
            p5_ts.append(pt_)

        # fc weights: 2 chunks on gpsimd (needed only after conv)
        w3_t = cpool.tile([100, 3600], f16, tag="w3")
        nc.gpsimd.dma_start(w3_t[:, 0:1800], w3_d[:, 0:1800])
        nc.gpsimd.dma_start(w3_t[:, 1800:3600], w3_d[:, 1800:3600])

        # hidden (101 rows: 100 hidden + ones row for the bias trick);
        # engine memset can't start at partition 100, so DMA the ones row
        hT_t = cpool.tile([101, B], f16, tag="hT")
        nc.scalar.dma_start(hT_t[100:101, :], ones_d[:])

        # conv: 2 groups (8 samples) per PSUM tile; evac alternates ACT/DVE
        X_t = cpool.tile([100, 36 * B], f16, tag="X")
        Xv = X_t[:].rearrange("p (hw s) -> p s hw", s=B)
        pconv = ctx.enter_context(tc.tile_pool(name="pconv", bufs=3, space="PSUM"))

        def chunk_of(g):
            for i, (g0, g1) in enumerate(CONV_CHUNKS):
                if g0 <= g < g1:
                    return i, g - g0
            raise ValueError(g)

        for t in range(NG // 2):
            pt = pconv.tile([100, 2 * GC], f32, tag="pconv")
            for j in (0, 1):
                g = 2 * t + j
                ci, loc = chunk_of(g)
                nc.tensor.matmul(
                    pt[:, j * GC:(j + 1) * GC],
                    r5_ts[ci][:, loc * 100:(loc + 1) * 100],
                    p5_ts[ci][:, loc * GC:(loc + 1) * GC],
                    start=True, stop=True)
            src = pt[:].rearrange("p (s hw) -> p s hw", hw=36)
            dst = Xv[:, 8 * t:8 * t + 8, :]
            if t % 2 == 0:
                nc.scalar.activation(dst, src, AF.Relu, bias=b1_t[:, 0:1])
            else:
                nc.vector.tensor_scalar(
                    dst, src, b1_t[:, 0:1], 0.0, OP.add, OP.max)

        # fc: accumulate 36 matmuls into one PSUM tile
        pfc_pool = ctx.enter_context(tc.tile_pool(name="pfc", bufs=1, space="PSUM"))
        pfc = pfc_pool.tile([100, B], f32, tag="pfc")
        for hw in range(36):
            nc.tensor.matmul(
                pfc[:],
                w3_t[:, hw * 100:(hw + 1) * 100],
                X_t[:, hw * B:(hw + 1) * B],
                start=(hw == 0), stop=(hw == 35))
        nc.scalar.activation(hT_t[0:100, :], pfc[:], AF.Relu, bias=b2_t[:, 0:1])

        # scoring: logits[m*128:+128, ci*500:+500] = hT_aug.T @ embT_aug
        psc = ctx.enter_context(tc.tile_pool(name="psc", bufs=4, space="PSUM"))
        sbp = ctx.enter_context(tc.tile_pool(name="sb", bufs=4))
        for m in range(B // 128):
            for ci in range(SH // NCH):
                it = m * (SH // NCH) + ci
                ps = psc.tile([128, NCH], f32, tag="psc")
                nc.tensor.matmul(
                    ps[:],
                    hT_t[:, m * 128:(m + 1) * 128],
                    embT_t[:, ci * NCH:(ci + 1) * NCH],
                    start=True, stop=True)
                sb = sbp.tile([128, NCH], f16, tag="sb")
                if it % 2 == 0:
                    nc.scalar.copy(sb[:], ps[:])
                else:
                    nc.vector.tensor_scalar(sb[:], ps[:], 0.0, None, OP.add)
                out_eng = (nc.sync, nc.gpsimd)[it % 2]
                out_eng.dma_start(
                    scores_d[m * 128:(m + 1) * 128, ci * NCH:(ci + 1) * NCH],
                    sb[:])

    nc.compile()
    return nc


def host_prep(inputs):
    f = {k: np.asarray(v) for k, v in inputs.items()}
    e1 = f['e1'].astype(np.int64)
    rel = f['rel'].astype(np.int64)
    e1e = np.ascontiguousarray(f['emb_e'][e1]).astype(np.float32)    # (B, 100)
    rg = np.ascontiguousarray(f['emb_rel'][rel]).astype(np.float32)  # (B, 2500)

    a0 = float(f['bn0_g'][0] / np.sqrt(f['bn0_v'][0] + EPS))
    b0 = float(f['bn0_b'][0] - f['bn0_m'][0] * a0)
    A1 = (f['bn1_g'] / np.sqrt(f['bn1_v'] + EPS)).astype(np.float32)
    B1 = (f['bn1_b'] - f['bn1_m'] * A1).astype(np.float32)
    s_rel = (f['bn_rel_g'] / np.sqrt(f['bn_rel_v'] + EPS)).astype(np.float32)
    t_rel = (f['bn_rel_b'] - f['bn_rel_m'] * s_rel).astype(np.float32)
    s_rel2 = s_rel * np.repeat(A1, 25)
    t_rel2 = t_rel * np.repeat(A1, 25)
    A2 = (f['bn2_g'] / np.sqrt(f['bn2_v'] + EPS)).astype(np.float32)
    B2p = ((f['fc_b'] - f['bn2_m']) * A2 + f['bn2_b']).astype(np.float32)

    # normalized, A1-folded filters -> r5[25s+k, 100g+c] = rn[4g+s, c, k]
    rn = (rg * s_rel2[None, :] + t_rel2[None, :]).reshape(B, 100, 25)
    r5 = np.empty((100, NG * 100), np.float16)
    for s in range(S):
        r5[25 * s:25 * s + 25] = (
            rn[s::S].transpose(2, 0, 1).reshape(25, NG * 100))

    # BN0-normalized patches -> block-diagonal p5:
    # p5[25s+k, 144g+36s'+hw] = (s==s') * patches[4g+s, k, hw]
    x0 = (e1e * a0 + b0).reshape(B, 10, 10)
    win = np.lib.stride_tricks.sliding_window_view(x0, (5, 5), axis=(1, 2))
    patches = win.transpose(0, 3, 4, 1, 2).reshape(B, 25, 36)  # (b, k, hw)
    p5 = np.zeros((100, NG * GC), np.float16)
    p5v = p5.reshape(S, 25, NG, S, 36)
    for s in range(S):
        p5v[s, :, :, s, :] = patches[s::S].transpose(1, 0, 2)

    w3 = np.ascontiguousarray(
        (f['fc_w'].astype(np.float32) * A2[None, :]).reshape(100, 3600)
    ).astype(np.float16)
    embT = np.concatenate(
        [f['emb_e'].T, f['bias'][None, :]], 0).astype(np.float16)  # (101, NE)

    col = lambda v: np.ascontiguousarray(v.reshape(100, 1)).astype(np.float32)
    common = dict(r5=r5, p5=p5, b1c=col(B1), w3=w3, b2c=col(B2p),
                  ones=np.ones((1, B), np.float16))
    in_maps = []
    for m in range(NCORES):
        d = dict(common)
        d['embT'] = np.ascontiguousarray(embT[:, m * SH:(m + 1) * SH])
        in_maps.append(d)
    return in_maps


def _get_nc():
    if 'nc' not in _CACHE:
        _CACHE['nc'] = _build()
    return _CACHE['nc']


def kernel(**inputs):
    from concourse import bass_utils
    from concourse.bass_interp import get_hw_module

    nc = _get_nc()
    in_maps = host_prep(inputs)

    kwargs = {}
    trace_dir = os.environ.get("CONVR_TRACE_DIR")
    if trace_dir:
        kwargs.update(tmpdir=trace_dir, trace=True)

    old_m = nc.m
    nc.m = get_hw_module(nc.m)
    try:
        res = bass_utils.run_bass_kernel_spmd(
            nc, in_maps, core_ids=list(range(NCORES)), **kwargs)
    finally:
        nc.m = old_m
    _CACHE['last_result'] = res

    logits = np.empty((B, NE), np.float32)
    for m in range(NCORES):
        logits[:, m * SH:(m + 1) * SH] = res.results[m]['scores']
    return (1.0 / (1.0 + np.exp(-logits))).astype(np.float32)


# revision 9
# speedup vs baseline: 3.0490x; 1.0952x over previous
"""ConvR (dense_cnn) Trainium2 kernel — 8-core vocab/tensor-parallel, fp16.

Strategy (per sharding hint): entity-embedding table and output scores are
column-sharded across the 8 cores; the small conv/fc path is replicated on
every core (each core computes the full 256-sample hidden, then scores its
12500-entity shard).

v2 redesign vs baseline (384us):
  - all matmul operands fp16 (PE streams 1 cyc/col vs 4 for fp32, single
    LDWEIGHTS pass); PSUM accumulation stays f32.
  - conv packs S=4 samples per matmul via block-diagonal rhs: partitions
    hold 4x25 patch rows, rhs [100, 144] has each sample's 36 patch columns
    in its own 25-partition band (zeros elsewhere), lhsT [100, 100] stacks
    the 4 samples' filters. 64 matmuls instead of 256.
  - input DMAs chunked across queues/engines (a single dma_start tends to
    pin one ~22.5 GB/s DMA engine; the baseline's monolithic embT load
    serialized 5 MB onto one engine for 195us).
  - PSUM evacuation split between scalar(ACT) and vector(DVE) — the only
    two engines with PSUM access.
  - scores leave the device as fp16 logits; sigmoid + f32 upcast on host.
"""
import os
import sys

sys.path.insert(0, "/opt/trn_rl_repo")

import numpy as np
from contextlib import ExitStack

B = 256          # batch
E = 100          # embedding dim
NE = 100000      # entities
NCORES = 8
SH = NE // NCORES   # 12500 entities per core
NCH = 500           # scoring N-chunk (one PSUM bank of f32)
S = 4               # conv samples packed per matmul (4*25=100 partitions)
NG = B // S         # 64 conv groups
GC = S * 36         # 144 rhs cols per conv group
EPS = 1e-5

# chunk boundaries (in conv groups) for r5/p5 input streaming: small first
# chunks so the first conv matmul can start early
CONV_CHUNKS = [(0, 4), (4, 8), (8, 16), (16, 32), (32, 64)]

_CACHE = {}


def _emb_row_chunks():
    # 101 rows -> 13 partition chunks (12x8 + 5)
    bounds = list(range(0, 96, 8)) + [96, 101]
    return list(zip(bounds[:-1], bounds[1:]))


def _build():
    import concourse.bass as bass  # noqa: F401
    import concourse.tile as tile
    from concourse import bacc, mybir

    f32 = mybir.dt.float32
    f16 = mybir.dt.float16
    AF = mybir.ActivationFunctionType
    OP = mybir.AluOpType

    nc = bacc.Bacc("TRN2", target_bir_lowering=False, debug=False,
                   num_devices=NCORES)

    r5_d = nc.dram_tensor("r5", [100, NG * 100], f16, kind="ExternalInput").ap()
    p5_d = nc.dram_tensor("p5", [100, NG * GC], f16, kind="ExternalInput").ap()
    b1_d = nc.dram_tensor("b1c", [100, 1], f32, kind="ExternalInput").ap()
    w3_d = nc.dram_tensor("w3", [100, 3600], f16, kind="ExternalInput").ap()
    b2_d = nc.dram_tensor("b2c", [100, 1], f32, kind="ExternalInput").ap()
    ones_d = nc.dram_tensor("ones", [1, B], f16, kind="ExternalInput").ap()
    embT_d = nc.dram_tensor("embT", [101, SH], f16, kind="ExternalInput").ap()
    scores_d = nc.dram_tensor("scores", [B, SH], f16, kind="ExternalOutput").ap()

    with tile.TileContext(nc) as tc, ExitStack() as ctx:
        cpool = ctx.enter_context(tc.tile_pool(name="const", bufs=1))

        # biases first (tiny, needed at first conv evac)
        b1_t = cpool.tile([100, 1], f32, tag="b1c")
        nc.scalar.dma_start(b1_t[:], b1_d[:])
        b2_t = cpool.tile([100, 1], f32, tag="b2c")
        nc.scalar.dma_start(b2_t[:], b2_d[:])

        # conv operands first on sync/gpsimd: separate tiles per chunk
        # (per-tile dep granularity so conv can start on chunk 0)
        r5_ts, p5_ts = [], []
        for i, (g0, g1) in enumerate(CONV_CHUNKS):
            rt = cpool.tile([100, (g1 - g0) * 100], f16, tag=f"r5_{i}")
            pt_ = cpool.tile([100, (g1 - g0) * GC], f16, tag=f"p5_{i}")
            qa, qb = (nc.sync, nc.gpsimd) if i % 2 == 0 else (nc.gpsimd, nc.sync)
            qa.dma_start(rt[:], r5_d[:, g0 * 100:g1 * 100])
            qb.dma_start(pt_[:], p5_d[:, g0 * GC:g1 * GC])
            r5_ts.append(rt)
            p5_ts.append(pt_)

        # embT: 26 row-chunks of ~4 rows (~100 KB each) spread over all three
        # DMA-issuing queues so many SDMA engines pull concurrently
        embT_t = cpool.tile([101, SH], f16, tag="embT")
        embq = (nc.scalar, nc.sync, nc.gpsimd)
        bounds = list(range(0, 101, 4)) + [101]
        for i, (r0, r1) in enumerate(zip(bounds[:-1], bounds[1:])):
            embq[i % 3].dma_start(embT_t[r0:r1, :], embT_d[r0:r1, :])

        # fc weights: 2 chunks on gpsimd (needed only after conv)
        w3_t = cpool.tile([100, 3600], f16, tag="w3")
        nc.gpsimd.dma_start(w3_t[:, 0:1800], w3_d[:, 0:1800])
        nc.gpsimd.dma_start(w3_t[:, 1800:3600], w3_d[:, 1800:3600])

        # hidden (101 rows: 100 hidden + ones row for the bias trick);
        # engine memset can't start at partition 100, so DMA the ones row
        hT_t = cpool.tile([101, B], f16, tag="hT")
        nc.scalar.dma_start(hT_t[100:101, :], ones_d[:])

        # conv: 6 groups (24 samples) per 2-bank PSUM tile [100, 1024]f32,
        # groups j at col 512*(j//3) + 144*(j%3); one batched evac per bank
        # (PSUM evac has ~600ns fixed cost per instruction — batch it)
        X_t = cpool.tile([100, 36 * B], f16, tag="X")
        Xv = X_t[:].rearrange("p (hw s) -> p s hw", s=B)

        def chunk_of(g):
            for i, (g0, g1) in enumerate(CONV_CHUNKS):
                if g0 <= g < g1:
                    return i, g - g0
            raise ValueError(g)

        evac_idx = 0
        with tc.tile_pool(name="pconv", bufs=2, space="PSUM") as pconv, \
             tc.tile_pool(name="pfc", bufs=1, space="PSUM") as pfc_pool:
            ntile = (NG + 5) // 6
            for t in range(ntile):
                ng = min(6, NG - 6 * t)
                pt = pconv.tile([100, 1024], f32, tag="pconv")
                for j in range(ng):
                    g = 6 * t + j
                    ci, loc = chunk_of(g)
                    off = 512 * (j // 3) + 144 * (j % 3)
                    nc.tensor.matmul(
                        pt[:, off:off + GC],
                        r5_ts[ci][:, loc * 100:(loc + 1) * 100],
                        p5_ts[ci][:, loc * GC:(loc + 1) * GC],
                        start=True, stop=True)
                for bk in (0, 1):
                    nbg = min(3, ng - 3 * bk)  # groups in this bank
                    if nbg <= 0:
                        break
                    ns = 4 * nbg               # samples in this bank
                    src = pt[:, 512 * bk:512 * bk + 36 * ns].rearrange(
                        "p (s hw) -> p s hw", hw=36)
                    s0 = 24 * t + 12 * bk
                    dst = Xv[:, s0:s0 + ns, :]
                    if evac_idx % 2 == 0:
                        nc.scalar.activation(dst, src, AF.Relu, bias=b1_t[:, 0:1])
                    else:
                        nc.vector.tensor_scalar(
                            dst, src, b1_t[:, 0:1], 0.0, OP.add, OP.max)
                    evac_idx += 1

            # fc: accumulate 36 matmuls into one PSUM tile
            pfc = pfc_pool.tile([100, B], f32, tag="pfc")
            for hw in range(36):
                nc.tensor.matmul(
                    pfc[:],
                    w3_t[:, hw * 100:(hw + 1) * 100],
                    X_t[:, hw * B:(hw + 1) * B],
                    start=(hw == 0), stop=(hw == 35))
            nc.scalar.activation(hT_t[0:100, :], pfc[:], AF.Relu, bias=b2_t[:, 0:1])

        # scoring: 4-bank PSUM tiles [128, 2048]f32, matmuls of 512 cols
        # (one full bank each — matmul output must not cross a bank), one
        # batched evac + one 512KB out-DMA per tile
        CT = 2048                      # entity cols per scoring tile
        tiles_per_m = (SH + CT - 1) // CT   # 6x2048 + 1x212
        with tc.tile_pool(name="psc", bufs=2, space="PSUM") as psc, \
             tc.tile_pool(name="sb", bufs=4) as sbp:
            it = 0
            for m in range(B // 128):
                for ti in range(tiles_per_m):
                    c0 = ti * CT
                    nct = min(CT, SH - c0)
                    ps = psc.tile([128, nct], f32, tag="psc")
                    for q in range((nct + 511) // 512):
                        nq = min(512, nct - q * 512)
                        nc.tensor.matmul(
                            ps[:, q * 512:q * 512 + nq],
                            hT_t[:, m * 128:(m + 1) * 128],
                            embT_t[:, c0 + q * 512:c0 + q * 512 + nq],
                            start=True, stop=True)
                    sb = sbp.tile([128, nct], f16, tag="sb")
                    if it % 2 == 0:
                        nc.scalar.copy(sb[:], ps[:])
                    else:
                        nc.vector.tensor_scalar(sb[:], ps[:], 0.0, None, OP.add)
                    out_eng = (nc.sync, nc.gpsimd)[it % 2]
                    out_eng.dma_start(
                        scores_d[m * 128:(m + 1) * 128, c0:c0 + nct], sb[:])
                    it += 1

    nc.compile()
    return nc


def host_prep(inputs):
    f = {k: np.asarray(v) for k, v in inputs.items()}
    e1 = f['e1'].astype(np.int64)
    rel = f['rel'].astype(np.int64)
    e1e = np.ascontiguousarray(f['emb_e'][e1]).astype(np.float32)    # (B, 100)
    rg = np.ascontiguousarray(f['emb_rel'][rel]).astype(np.float32)  # (B, 2500)

    a0 = float(f['bn0_g'][0] / np.sqrt(f['bn0_v'][0] + EPS))
    b0 = float(f['bn0_b'][0] - f['bn0_m'][0] * a0)
    A1 = (f['bn1_g'] / np.sqrt(f['bn1_v'] + EPS)).astype(np.float32)
    B1 = (f['bn1_b'] - f['bn1_m'] * A1).astype(np.float32)
    s_rel = (f['bn_rel_g'] / np.sqrt(f['bn_rel_v'] + EPS)).astype(np.float32)
    t_rel = (f['bn_rel_b'] - f['bn_rel_m'] * s_rel).astype(np.float32)
    s_rel2 = s_rel * np.repeat(A1, 25)
    t_rel2 = t_rel * np.repeat(A1, 25)
    A2 = (f['bn2_g'] / np.sqrt(f['bn2_v'] + EPS)).astype(np.float32)
    B2p = ((f['fc_b'] - f['bn2_m']) * A2 + f['bn2_b']).astype(np.float32)

    # normalized, A1-folded filters -> r5[25s+k, 100g+c] = rn[4g+s, c, k]
    rn = (rg * s_rel2[None, :] + t_rel2[None, :]).reshape(B, 100, 25)
    r5 = np.empty((100, NG * 100), np.float16)
    for s in range(S):
        r5[25 * s:25 * s + 25] = (
            rn[s::S].transpose(2, 0, 1).reshape(25, NG * 100))

    # BN0-normalized patches -> block-diagonal p5:
    # p5[25s+k, 144g+36s'+hw] = (s==s') * patches[4g+s, k, hw]
    x0 = (e1e * a0 + b0).reshape(B, 10, 10)
    win = np.lib.stride_tricks.sliding_window_view(x0, (5, 5), axis=(1, 2))
    patches = win.transpose(0, 3, 4, 1, 2).reshape(B, 25, 36)  # (b, k, hw)
    p5 = np.zeros((100, NG * GC), np.float16)
    p5v = p5.reshape(S, 25, NG, S, 36)
    for s in range(S):
        p5v[s, :, :, s, :] = patches[s::S].transpose(1, 0, 2)

    w3 = np.ascontiguousarray(
        (f['fc_w'].astype(np.float32) * A2[None, :]).reshape(100, 3600)
    ).astype(np.float16)
    embT = np.concatenate(
        [f['emb_e'].T, f['bias'][None, :]], 0).astype(np.float16)  # (101, NE)

    col = lambda v: np.ascontiguousarray(v.reshape(100, 1)).astype(np.float32)
    common = dict(r5=r5, p5=p5, b1c=col(B1), w3=w3, b2c=col(B2p),
                  ones=np.ones((1, B), np.float16))
    in_maps = []
    for m in range(NCORES):
        d = dict(common)
        d['embT'] = np.ascontiguousarray(embT[:, m * SH:(m + 1) * SH])
        in_maps.append(d)
    return in_maps


def _get_nc():
    if 'nc' not in _CACHE:
        _CACHE['nc'] = _build()
    return _CACHE['nc']


def kernel(**inputs):
    from concourse import bass_utils
    from concourse.bass_interp import get_hw_module

    nc = _get_nc()
    in_maps = host_prep(inputs)

    kwargs = {}
    trace_dir = os.environ.get("CONVR_TRACE_DIR")
    if trace_dir:
        kwargs.update(tmpdir=trace_dir, trace=True)

    old_m = nc.m
    nc.m = get_hw_module(nc.m)
    try:
        res = bass_utils.run_bass_kernel_spmd(
            nc, in_maps, core_ids=list(range(NCORES)), **kwargs)
    finally:
        nc.m = old_m
    _CACHE['last_result'] = res

    logits = np.empty((B, NE), np.float32)
    for m in range(NCORES):
        logits[:, m * SH:(m + 1) * SH] = res.results[m]['scores']
    return (1.0 / (1.0 + np.exp(-logits))).astype(np.float32)


# revision 13
# speedup vs baseline: 3.7848x; 1.2413x over previous
"""ConvR (dense_cnn) Trainium2 kernel — 8-core vocab/tensor-parallel, fp16.

Strategy (per sharding hint): entity-embedding table and output scores are
column-sharded across the 8 cores; the small conv/fc path is replicated on
every core (each core computes the full 256-sample hidden, then scores its
12500-entity shard).

v2 redesign vs baseline (384us):
  - all matmul operands fp16 (PE streams 1 cyc/col vs 4 for fp32, single
    LDWEIGHTS pass); PSUM accumulation stays f32.
  - conv packs S=4 samples per matmul via block-diagonal rhs: partitions
    hold 4x25 patch rows, rhs [100, 144] has each sample's 36 patch columns
    in its own 25-partition band (zeros elsewhere), lhsT [100, 100] stacks
    the 4 samples' filters. 64 matmuls instead of 256.
  - input DMAs chunked across queues/engines (a single dma_start tends to
    pin one ~22.5 GB/s DMA engine; the baseline's monolithic embT load
    serialized 5 MB onto one engine for 195us).
  - PSUM evacuation split between scalar(ACT) and vector(DVE) — the only
    two engines with PSUM access.
  - scores leave the device as fp16 logits; sigmoid + f32 upcast on host.
"""
import os
import sys

sys.path.insert(0, "/opt/trn_rl_repo")

import numpy as np
from contextlib import ExitStack

B = 256          # batch
E = 100          # embedding dim
NE = 100000      # entities
NCORES = 8
SH = NE // NCORES   # 12500 entities per core
NCH = 500           # scoring N-chunk (one PSUM bank of f32)
S = 4               # conv samples packed per matmul (4*25=100 partitions)
NG = B // S         # 64 conv groups
GC = S * 36         # 144 rhs cols per conv group
EPS = 1e-5

# chunk boundaries (in conv groups) for r5/p5 input streaming: small first
# chunk so the first conv matmul can start early
CONV_CHUNKS = [(0, 8), (8, 32), (32, 64)]

_CACHE = {}


def _emb_row_chunks():
    # 101 rows -> 8 partition chunks (7x13 + 10)
    bounds = list(range(0, 92, 13)) + [101]
    return list(zip(bounds[:-1], bounds[1:]))


def _build():
    import concourse.bass as bass  # noqa: F401
    import concourse.tile as tile
    from concourse import bacc, mybir

    f32 = mybir.dt.float32
    f16 = mybir.dt.float16
    AF = mybir.ActivationFunctionType
    OP = mybir.AluOpType

    nc = bacc.Bacc("TRN2", target_bir_lowering=False, debug=False,
                   num_devices=NCORES)

    r5_d = nc.dram_tensor("r5", [100, NG * 100], f16, kind="ExternalInput").ap()
    p5_d = nc.dram_tensor("p5", [100, NG * GC], f16, kind="ExternalInput").ap()
    b1_d = nc.dram_tensor("b1c", [100, 1], f32, kind="ExternalInput").ap()
    w3_d = nc.dram_tensor("w3", [100, 3600], f16, kind="ExternalInput").ap()
    b2_d = nc.dram_tensor("b2c", [100, 1], f32, kind="ExternalInput").ap()
    ones_d = nc.dram_tensor("ones", [1, B], f16, kind="ExternalInput").ap()
    embT_d = nc.dram_tensor("embT", [101, SH], f16, kind="ExternalInput").ap()
    scores_d = nc.dram_tensor("scores", [B, SH], f16, kind="ExternalOutput").ap()

    with tile.TileContext(nc) as tc, ExitStack() as ctx:
        cpool = ctx.enter_context(tc.tile_pool(name="const", bufs=1))

        # biases first (tiny, needed at first conv evac)
        b1_t = cpool.tile([100, 1], f32, tag="b1c")
        nc.scalar.dma_start(b1_t[:], b1_d[:])
        b2_t = cpool.tile([100, 1], f32, tag="b2c")
        nc.scalar.dma_start(b2_t[:], b2_d[:])

        # ALL bulk input DMAs go on sync/gpsimd only: dma_start instructions
        # carry semaphore-reuse waits that stall the issuing engine, so the
        # compute engines (scalar/vector) must never issue long DMA chains.
        # Conv chunks first (they gate conv->fc->scoring), then w3/embT.
        r5_ts, p5_ts = [], []
        for i, (g0, g1) in enumerate(CONV_CHUNKS):
            rt = cpool.tile([100, (g1 - g0) * 100], f16, tag=f"r5_{i}")
            pt_ = cpool.tile([100, (g1 - g0) * GC], f16, tag=f"p5_{i}")
            qa, qb = (nc.sync, nc.gpsimd) if i % 2 == 0 else (nc.gpsimd, nc.sync)
            qa.dma_start(rt[:], r5_d[:, g0 * 100:g1 * 100])
            qb.dma_start(pt_[:], p5_d[:, g0 * GC:g1 * GC])
            r5_ts.append(rt)
            p5_ts.append(pt_)

        w3_t = cpool.tile([100, 3600], f16, tag="w3")
        nc.gpsimd.dma_start(w3_t[:], w3_d[:])

        embT_t = cpool.tile([101, SH], f16, tag="embT")
        for i, (r0, r1) in enumerate(_emb_row_chunks()):
            eng = nc.sync if i % 2 == 0 else nc.gpsimd
            eng.dma_start(embT_t[r0:r1, :], embT_d[r0:r1, :])

        # hidden (101 rows: 100 hidden + ones row for the bias trick);
        # engine memset can't start at partition 100, so DMA the ones row
        hT_t = cpool.tile([101, B], f16, tag="hT")
        nc.scalar.dma_start(hT_t[100:101, :], ones_d[:])

        # conv: 6 groups (24 samples) per 2-bank PSUM tile [100, 1024]f32,
        # group j at col 512*(j//3) + 144*(j%3). X is SAMPLE-major
        # (col = b*36 + hw) so evac writes are contiguous — strided engine
        # writes cost ~5 ns/elem vs 0.83 contiguous; fc instead takes the
        # stride on its matmul rhs which streams at full rate.
        X_t = cpool.tile([100, 36 * B], f16, tag="X")
        Xhw = X_t[:].rearrange("p (b hw) -> p hw b", hw=36)

        def chunk_of(g):
            for i, (g0, g1) in enumerate(CONV_CHUNKS):
                if g0 <= g < g1:
                    return i, g - g0
            raise ValueError(g)

        evac_idx = 0
        with tc.tile_pool(name="pconv", bufs=2, space="PSUM") as pconv, \
             tc.tile_pool(name="pfc", bufs=1, space="PSUM") as pfc_pool:
            ntile = (NG + 5) // 6
            for t in range(ntile):
                ng = min(6, NG - 6 * t)
                pt = pconv.tile([100, 1024], f32, tag="pconv")
                for j in range(ng):
                    g = 6 * t + j
                    ci, loc = chunk_of(g)
                    off = 512 * (j // 3) + 144 * (j % 3)
                    nc.tensor.matmul(
                        pt[:, off:off + GC],
                        r5_ts[ci][:, loc * 100:(loc + 1) * 100],
                        p5_ts[ci][:, loc * GC:(loc + 1) * GC],
                        start=True, stop=True)
                n0 = 144 * min(3, ng)               # cols in bank 0
                n1 = 144 * max(0, ng - 3)           # cols in bank 1
                c0 = 24 * t * 36                    # dst col of first sample
                if n0 == n1:
                    # one 3D->3D evac covering both banks
                    src = pt[:].rearrange("p (bk x) -> p bk x", bk=2)[:, :, 0:n0]
                    dst = X_t[:, c0:c0 + 2 * n0].rearrange(
                        "p (bk x) -> p bk x", bk=2)
                    pairs = [(src, dst)]
                else:
                    pairs = [(pt[:, 0:n0], X_t[:, c0:c0 + n0])]
                    if n1:
                        pairs.append((pt[:, 512:512 + n1],
                                      X_t[:, c0 + n0:c0 + n0 + n1]))
                for src, dst in pairs:
                    if evac_idx % 2 == 0:
                        nc.scalar.activation(dst, src, AF.Relu, bias=b1_t[:, 0:1])
                    else:
                        nc.vector.tensor_scalar(
                            dst, src, b1_t[:, 0:1], 0.0, OP.add, OP.max)
                    evac_idx += 1

            # fc: accumulate 36 matmuls into one PSUM tile; rhs is X strided
            # by hw (sample-major X), streamed at 1 col/cycle regardless
            pfc = pfc_pool.tile([100, B], f32, tag="pfc")
            for hw in range(36):
                nc.tensor.matmul(
                    pfc[:],
                    w3_t[:, hw * 100:(hw + 1) * 100],
                    Xhw[:, hw, :],
                    start=(hw == 0), stop=(hw == 35))
            nc.scalar.activation(hT_t[0:100, :], pfc[:], AF.Relu, bias=b2_t[:, 0:1])

        # scoring: 2-bank PSUM tiles [128, 1024]f32 with bufs=4 (deep
        # matmul/evac/DMA pipeline), matmuls of 512 cols (one full bank each
        # — matmul output must not cross a bank), one batched evac + one
        # 256KB out-DMA per tile
        CT = 1024                      # entity cols per scoring tile
        tiles_per_m = (SH + CT - 1) // CT   # 12x1024 + 1x212
        with tc.tile_pool(name="psc", bufs=4, space="PSUM") as psc, \
             tc.tile_pool(name="sb", bufs=6) as sbp:
            it = 0
            for m in range(B // 128):
                for ti in range(tiles_per_m):
                    c0 = ti * CT
                    nct = min(CT, SH - c0)
                    ps = psc.tile([128, nct], f32, tag="psc")
                    for q in range((nct + 511) // 512):
                        nq = min(512, nct - q * 512)
                        nc.tensor.matmul(
                            ps[:, q * 512:q * 512 + nq],
                            hT_t[:, m * 128:(m + 1) * 128],
                            embT_t[:, c0 + q * 512:c0 + q * 512 + nq],
                            start=True, stop=True)
                    sb = sbp.tile([128, nct], f16, tag="sb")
                    if it % 2 == 0:
                        nc.scalar.copy(sb[:], ps[:])
                    else:
                        nc.vector.tensor_scalar(sb[:], ps[:], 0.0, None, OP.add)
                    out_eng = (nc.sync, nc.gpsimd)[it % 2]
                    out_eng.dma_start(
                        scores_d[m * 128:(m + 1) * 128, c0:c0 + nct], sb[:])
                    it += 1

    nc.compile()
    return nc


def host_prep(inputs):
    f = {k: np.asarray(v) for k, v in inputs.items()}
    e1 = f['e1'].astype(np.int64)
    rel = f['rel'].astype(np.int64)
    e1e = np.ascontiguousarray(f['emb_e'][e1]).astype(np.float32)    # (B, 100)
    rg = np.ascontiguousarray(f['emb_rel'][rel]).astype(np.float32)  # (B, 2500)

    a0 = float(f['bn0_g'][0] / np.sqrt(f['bn0_v'][0] + EPS))
    b0 = float(f['bn0_b'][0] - f['bn0_m'][0] * a0)
    A1 = (f['bn1_g'] / np.sqrt(f['bn1_v'] + EPS)).astype(np.float32)
    B1 = (f['bn1_b'] - f['bn1_m'] * A1).astype(np.float32)
    s_rel = (f['bn_rel_g'] / np.sqrt(f['bn_rel_v'] + EPS)).astype(np.float32)
    t_rel = (f['bn_rel_b'] - f['bn_rel_m'] * s_rel).astype(np.float32)
    s_rel2 = s_rel * np.repeat(A1, 25)
    t_rel2 = t_rel * np.repeat(A1, 25)
    A2 = (f['bn2_g'] / np.sqrt(f['bn2_v'] + EPS)).astype(np.float32)
    B2p = ((f['fc_b'] - f['bn2_m']) * A2 + f['bn2_b']).astype(np.float32)

    # normalized, A1-folded filters -> r5[25s+k, 100g+c] = rn[4g+s, c, k]
    rn = (rg * s_rel2[None, :] + t_rel2[None, :]).reshape(B, 100, 25)
    r5 = np.empty((100, NG * 100), np.float16)
    for s in range(S):
        r5[25 * s:25 * s + 25] = (
            rn[s::S].transpose(2, 0, 1).reshape(25, NG * 100))

    # BN0-normalized patches -> block-diagonal p5:
    # p5[25s+k, 144g+36s'+hw] = (s==s') * patches[4g+s, k, hw]
    x0 = (e1e * a0 + b0).reshape(B, 10, 10)
    win = np.lib.stride_tricks.sliding_window_view(x0, (5, 5), axis=(1, 2))
    patches = win.transpose(0, 3, 4, 1, 2).reshape(B, 25, 36)  # (b, k, hw)
    p5 = np.zeros((100, NG * GC), np.float16)
    p5v = p5.reshape(S, 25, NG, S, 36)
    for s in range(S):
        p5v[s, :, :, s, :] = patches[s::S].transpose(1, 0, 2)

    w3 = np.ascontiguousarray(
        (f['fc_w'].astype(np.float32) * A2[None, :]).reshape(100, 3600)
    ).astype(np.float16)
    embT = np.concatenate(
        [f['emb_e'].T, f['bias'][None, :]], 0).astype(np.float16)  # (101, NE)

    col = lambda v: np.ascontiguousarray(v.reshape(100, 1)).astype(np.float32)
    common = dict(r5=r5, p5=p5, b1c=col(B1), w3=w3, b2c=col(B2p),
                  ones=np.ones((1, B), np.float16))
    in_maps = []
    for m in range(NCORES):
        d = dict(common)
        d['embT'] = np.ascontiguousarray(embT[:, m * SH:(m + 1) * SH])
        in_maps.append(d)
    return in_maps


def _get_nc():
    if 'nc' not in _CACHE:
        _CACHE['nc'] = _build()
    return _CACHE['nc']


def kernel(**inputs):
    from concourse import bass_utils
    from concourse.bass_interp import get_hw_module

    nc = _get_nc()
    in_maps = host_prep(inputs)

    kwargs = {}
    trace_dir = os.environ.get("CONVR_TRACE_DIR")
    if trace_dir:
        kwargs.update(tmpdir=trace_dir, trace=True)

    old_m = nc.m
    nc.m = get_hw_module(nc.m)
    try:
        res = bass_utils.run_bass_kernel_spmd(
            nc, in_maps, core_ids=list(range(NCORES)), **kwargs)
    finally:
        nc.m = old_m
    _CACHE['last_result'] = res

    logits = np.empty((B, NE), np.float32)
    for m in range(NCORES):
        logits[:, m * SH:(m + 1) * SH] = res.results[m]['scores']
    return (1.0 / (1.0 + np.exp(-logits))).astype(np.float32)
